# revision 4
# baseline (speedup 1.0000x reference)
"""Trainium2 Bass kernel for nn_CrossBaby_1 (B=32, S=128, V=8192, E=256).

Strategy (8 NeuronCores, single NEFF, collectives):
  - Step 1 (x @ w_emb.T, the 17 GFLOP matmul): data-parallel over batch.
    Each core computes hT for its 4 batches from a host-pretransposed,
    bf16-cast x shard. PSUM-accumulated over 64 K-chunks of V.
  - AllGather of hT (bf16, 256KB/core) + per-batch row sums s.
  - Steps 3-5 (w_red / w_red2, the 67MB of weights): tensor-parallel over
    the e/j feature dim — each core holds 1/8 of w_red and w_red2 and
    processes ALL 32 batches for its feature shard.
  - AllReduce of the partial y2 (32x256 f32).
  - Step 6 (w_out): tensor-parallel over vocab; each core emits
    out[:, c*1024:(c+1)*1024]; host concatenates.
  All matmul operands bf16 (fp32 PSUM accumulation); biases/activations fp32.
"""

import numpy as np
import ml_dtypes

B, S, V, E = 32, 128, 8192, 256
NC = 8
BL = B // NC    # 4 local batches
ES = E // NC    # 32 feature shard (steps 3-5)
VS = V // NC    # 1024 vocab shard (step 6)
NCOL = BL * S   # 512 columns of local hT
GHT = 2 * 128 * NCOL          # bf16 elements of hT in gather payload
GLEN = GHT + 128 * BL         # + flattened s

_CACHE: dict = {}


def _build_nc():
    import concourse.bacc as bacc
    import concourse.mybir as mybir
    import concourse.tile as tile

    bf = mybir.dt.bfloat16
    f32 = mybir.dt.float32
    AF = mybir.ActivationFunctionType
    ALU = mybir.AluOpType

    nc = bacc.Bacc("TRN2", target_bir_lowering=False, debug=False, num_devices=NC)

    xt = nc.dram_tensor("xt", [V, NCOL], bf, kind="ExternalInput")
    wembT = nc.dram_tensor("wembT", [V, E], bf, kind="ExternalInput")
    bemb = nc.dram_tensor("bemb", [E], f32, kind="ExternalInput")
    wrT = nc.dram_tensor("wrT", [ES, S, E], bf, kind="ExternalInput")
    bredrep = nc.dram_tensor("bredrep", [S, ES], f32, kind="ExternalInput")
    w2p = nc.dram_tensor("w2p", [ES, S, E], bf, kind="ExternalInput")
    bred2 = nc.dram_tensor("bred2", [E], f32, kind="ExternalInput")
    woT = nc.dram_tensor("woT", [E, VS], bf, kind="ExternalInput")
    boutrep = nc.dram_tensor("boutrep", [B, VS], f32, kind="ExternalInput")
    ones = nc.dram_tensor("ones", [S, 1], bf, kind="ExternalInput")
    ident = nc.dram_tensor("ident", [B, B], f32, kind="ExternalInput")
    out_ext = nc.dram_tensor("out", [B, VS], f32, kind="ExternalOutput")

    gin = nc.dram_tensor("gin", [GLEN], bf)
    gout = nc.dram_tensor("gout", [NC, GLEN], bf, addr_space="Shared")
    arin = nc.dram_tensor("arin", [B, E], f32)
    arout = nc.dram_tensor("arout", [B, E], f32, addr_space="Shared")

    groups = [list(range(NC))]

    with tile.TileContext(nc) as tc:
        with (
            tc.tile_pool(name="persist", bufs=1) as pp,
            tc.tile_pool(name="xload", bufs=4) as xpool,
            tc.tile_pool(name="weload", bufs=4) as wepool,
        ):
            # ---------- persistent SBUF ----------
            hT_all = pp.tile([128, 2 * B * S], bf)       # [j128, (jc, b, s)]
            sT_all = pp.tile([128, B], bf)               # [k, (c,b)]
            weff = pp.tile([128, 2 * ES * B], bf)        # [j128, (jc, e, b)]
            y1 = pp.tile([128, B * ES], bf)              # [k, (b, j)]
            hsb = pp.tile([128, 2 * NCOL], bf)           # local hT [j128,(jc,n)]
            s_bf = pp.tile([1, NCOL], bf)
            bemb_sb = pp.tile([128, 2], f32)
            bredrep_sb = pp.tile([128, ES], f32)
            bred2_sb = pp.tile([128, 2], f32)
            ones_sb = pp.tile([128, 1], bf)
            ident_sb = pp.tile([B, B], f32)
            y2p_sb = pp.tile([B, E], f32)
            y2r_sb = pp.tile([B, E], f32)
            y2T = pp.tile([128, 2 * B], bf)              # [e128, (ec, b)]
            wo_sb = pp.tile([128, 2 * VS], bf)           # [e128, (ec, v)]
            boutrep_sb = pp.tile([B, VS], f32)
            outsb = pp.tile([B, VS], f32)

            nc.sync.dma_start(bemb_sb[:, :], bemb.ap().rearrange("(ec p) -> p ec", p=128))
            nc.sync.dma_start(bredrep_sb[:, :], bredrep[:, :])
            nc.sync.dma_start(bred2_sb[:, :], bred2.ap().rearrange("(ec p) -> p ec", p=128))
            nc.sync.dma_start(ones_sb[:, :], ones[:, :])
            nc.sync.dma_start(ident_sb[:, :], ident[:, :])
            nc.sync.dma_start(boutrep_sb[:, :], boutrep[:, :])
            nc.sync.dma_start(
                wo_sb.rearrange("p (ec v) -> p ec v", ec=2),
                woT.ap().rearrange("(ec p) v -> p ec v", p=128),
            )

            # ---------- phase 1: hT = relu(w_embT.T @ xT + b_emb) ----------
            with tc.tile_pool(name="psum1", bufs=1, space="PSUM") as pp1:
                ph0 = pp1.tile([128, NCOL], f32)
                ph1 = pp1.tile([128, NCOL], f32)
                ps = pp1.tile([1, NCOL], f32)
                phs = [ph0, ph1]
                NV = V // 128
                for vc in range(NV):
                    xt_t = xpool.tile([128, NCOL], bf, tag="xt")
                    nc.sync.dma_start(xt_t[:, :], xt[vc * 128:(vc + 1) * 128, :])
                    we_t = wepool.tile([128, E], bf, tag="we")
                    nc.sync.dma_start(we_t[:, :], wembT[vc * 128:(vc + 1) * 128, :])
                    for ec in range(2):
                        nc.tensor.matmul(
                            phs[ec][:, :],
                            we_t[:, ec * 128:(ec + 1) * 128],
                            xt_t[:, :],
                            start=(vc == 0),
                            stop=(vc == NV - 1),
                        )
                for ec in range(2):
                    nc.scalar.activation(
                        hsb[:, ec * NCOL:(ec + 1) * NCOL],
                        phs[ec][:, :],
                        AF.Relu,
                        bias=bemb_sb[:, ec:ec + 1],
                    )
                # s = column sums of hT (over both j-chunks)
                for ec in range(2):
                    nc.tensor.matmul(
                        ps[:, :],
                        ones_sb[:, 0:1],
                        hsb[:, ec * NCOL:(ec + 1) * NCOL],
                        start=(ec == 0),
                        stop=(ec == 1),
                    )
                nc.vector.tensor_copy(s_bf[:, :], ps[:, :])

            # ---------- gather hT + s across cores ----------
            nc.sync.dma_start(
                gin.ap()[0:GHT].rearrange("(jc p n) -> p jc n", jc=2, p=128),
                hsb.rearrange("p (jc n) -> p jc n", jc=2),
            )
            nc.sync.dma_start(
                gin.ap()[GHT:GLEN].rearrange("(one n) -> one n", one=1),
                s_bf[:, :],
            )
            nc.gpsimd.collective_compute(
                "AllGather", ALU.bypass, groups,
                ins=[gin.ap().opt()], outs=[gout.ap().opt()],
            )
            for jc in range(2):
                nc.sync.dma_start(
                    hT_all[:, jc * B * S:(jc + 1) * B * S].rearrange(
                        "p (c n) -> p c n", c=NC
                    ),
                    gout.ap()[:, jc * 128 * NCOL:(jc + 1) * 128 * NCOL].rearrange(
                        "c (p n) -> p c n", p=128
                    ),
                )
            for c in range(NC):
                nc.sync.dma_start(
                    sT_all[:, c * BL:(c + 1) * BL],
                    gout.ap()[c, GHT:GLEN].rearrange("(b k) -> k b", b=BL),
                )

            weff_v = weff.rearrange("p (jc e b) -> p jc e b", jc=2, e=ES)
            y1_v = y1.rearrange("p (b j) -> p b j", b=B)

            with tc.tile_pool(name="psum2", bufs=1, space="PSUM") as pp2:
                # ---------- step 3: Weff[b, e, j] = sum_k Wr[e,k,j] s[b,k] ----------
                with tc.tile_pool(name="wrload", bufs=4) as wrpool:
                    for el in range(ES):
                        wr_t = wrpool.tile([128, E], bf, tag="wr")
                        nc.sync.dma_start(wr_t[:, :], wrT.ap()[el])
                        for jc in range(2):
                            psw = pp2.tile([128, ES], f32, tag="p32", bufs=2)
                            nc.tensor.matmul(
                                psw[:, :],
                                wr_t[:, jc * 128:(jc + 1) * 128],
                                sT_all[:, :],
                            )
                            nc.vector.tensor_copy(weff_v[:, jc, el, :], psw[:, :])

                # ---------- step 4: y1 = relu(h @ Weff_b^T + b_red) ----------
                for b in range(B):
                    psy = pp2.tile([128, ES], f32, tag="p32b", bufs=2)
                    for jc in range(2):
                        nc.tensor.matmul(
                            psy[:, :],
                            hT_all[:, jc * B * S + b * S: jc * B * S + (b + 1) * S],
                            weff_v[:, jc, :, b],
                            start=(jc == 0),
                            stop=(jc == 1),
                        )
                    nc.vector.tensor_tensor(psy[:, :], psy[:, :], bredrep_sb[:, :], ALU.add)
                    nc.scalar.activation(y1_v[:, b, :], psy[:, :], AF.Relu)

                # ---------- step 5: y2p[b, eo] = sum_{k, j in shard} ----------
                py2 = pp2.tile([B, E], f32)
                with tc.tile_pool(name="w2load", bufs=4) as w2pool:
                    for jl in range(ES):
                        w2_t = w2pool.tile([128, E], bf, tag="w2")
                        nc.sync.dma_start(w2_t[:, :], w2p.ap()[jl])
                        nc.tensor.matmul(
                            py2[:, :],
                            y1_v[:, :, jl],
                            w2_t[:, :],
                            start=(jl == 0),
                            stop=(jl == ES - 1),
                        )
                nc.vector.tensor_copy(y2p_sb[:, :], py2[:, :])

                # ---------- all-reduce partial y2 ----------
                nc.sync.dma_start(arin[:, :], y2p_sb[:, :])
                nc.gpsimd.collective_compute(
                    "AllReduce", ALU.add, groups,
                    ins=[arin.ap().opt()], outs=[arout.ap().opt()],
                )
                nc.sync.dma_start(y2r_sb[:, :], arout[:, :])

                # ---------- y2T = relu(y2 + b_red2)^T ----------
                for ec in range(2):
                    pst = pp2.tile([128, B], f32, tag="p32", bufs=2)
                    nc.tensor.transpose(
                        pst[:, :], y2r_sb[:, ec * 128:(ec + 1) * 128], ident_sb[:, :]
                    )
                    nc.scalar.activation(
                        y2T[:, ec * B:(ec + 1) * B],
                        pst[:, :],
                        AF.Relu,
                        bias=bred2_sb[:, ec:ec + 1],
                    )

                # ---------- step 6: out = y2 @ w_out^T + b_out ----------
                for nv in range(2):
                    pso = pp2.tile([B, 512], f32, tag="po", bufs=2)
                    for ec in range(2):
                        nc.tensor.matmul(
                            pso[:, :],
                            y2T[:, ec * B:(ec + 1) * B],
                            wo_sb[:, ec * VS + nv * 512: ec * VS + (nv + 1) * 512],
                            start=(ec == 0),
                            stop=(ec == 1),
                        )
                    nc.vector.tensor_tensor(
                        outsb[:, nv * 512:(nv + 1) * 512],
                        pso[:, :],
                        boutrep_sb[:, nv * 512:(nv + 1) * 512],
                        ALU.add,
                    )
            nc.sync.dma_start(out_ext[:, :], outsb[:, :])

    nc.compile()
    return nc


def _get_nc():
    if "nc" not in _CACHE:
        _CACHE["nc"] = _build_nc()
    return _CACHE["nc"]


def _pack_inputs(x, w_emb, b_emb, w_red, b_red, w_red2, b_red2, w_out, b_out):
    bf = ml_dtypes.bfloat16
    f32 = np.float32
    wembT = np.ascontiguousarray(w_emb.T).astype(bf)                 # [V, E]
    Wr = np.asarray(w_red).reshape(E, S, E)                          # [e, k, j]
    W2 = np.asarray(w_red2).reshape(E, S, E)                         # [eo, k, j]
    woT = np.ascontiguousarray(np.asarray(w_out).T)                  # [E, V]
    bemb = np.ascontiguousarray(b_emb).astype(f32)
    bred2 = np.ascontiguousarray(b_red2).astype(f32)
    ones = np.ones((S, 1), dtype=bf)
    ident = np.eye(B, dtype=f32)

    in_maps = []
    for c in range(NC):
        xs = np.asarray(x[c * BL:(c + 1) * BL])                      # [4, S, V]
        xt = np.ascontiguousarray(xs.transpose(2, 0, 1).reshape(V, NCOL)).astype(bf)
        wrT_c = np.ascontiguousarray(Wr[c * ES:(c + 1) * ES]).astype(bf)      # [el,k,j]
        w2p_c = np.ascontiguousarray(
            W2[:, :, c * ES:(c + 1) * ES].transpose(2, 1, 0)
        ).astype(bf)                                                  # [jl,k,eo]
        woT_c = np.ascontiguousarray(woT[:, c * VS:(c + 1) * VS]).astype(bf)  # [E,VS]
        bredrep = np.ascontiguousarray(
            np.broadcast_to(b_red[c * ES:(c + 1) * ES], (S, ES))
        ).astype(f32)
        boutrep = np.ascontiguousarray(
            np.broadcast_to(b_out[c * VS:(c + 1) * VS], (B, VS))
        ).astype(f32)
        in_maps.append({
            "xt": xt, "wembT": wembT, "bemb": bemb,
            "wrT": wrT_c, "bredrep": bredrep,
            "w2p": w2p_c, "bred2": bred2,
            "woT": woT_c, "boutrep": boutrep,
            "ones": ones, "ident": ident,
        })
    return in_maps


def kernel(x, w_emb, b_emb, w_red, b_red, w_red2, b_red2, w_out, b_out):
    from concourse.bass_utils import run_bass_kernel_spmd

    nc = _get_nc()
    in_maps = _pack_inputs(x, w_emb, b_emb, w_red, b_red, w_red2, b_red2, w_out, b_out)
    res = run_bass_kernel_spmd(nc, in_maps, core_ids=list(range(NC)))
    out = np.concatenate([res.results[c]["out"] for c in range(NC)], axis=1)
    return np.ascontiguousarray(out, dtype=np.float32)


# revision 7
# speedup vs baseline: 9760.8441x; 9760.8441x over previous
"""Trainium2 Bass kernel for nn_CrossBaby_1 (B=32, S=128, V=8192, E=256).

Strategy (8 NeuronCores, single NEFF, collectives):
  - Step 1 (x @ w_emb.T, the 17 GFLOP matmul): data-parallel over batch.
    Each core computes hT for its 4 batches from a host-pretransposed,
    bf16-cast x shard. PSUM-accumulated over 64 K-chunks of V.
  - AllGather of hT (bf16, 256KB/core) + per-batch row sums s.
  - Steps 3-5 (w_red / w_red2, the 67MB of weights): tensor-parallel over
    the e/j feature dim — each core holds 1/8 of w_red and w_red2 and
    processes ALL 32 batches for its feature shard.
  - AllReduce of the partial y2 (32x256 f32).
  - Step 6 (w_out): tensor-parallel over vocab; each core emits
    out[:, c*1024:(c+1)*1024]; host concatenates.
  All matmul operands bf16 (fp32 PSUM accumulation); biases/activations fp32.
"""

import numpy as np
import ml_dtypes

B, S, V, E = 32, 128, 8192, 256
NC = 8
BL = B // NC    # 4 local batches
ES = E // NC    # 32 feature shard (steps 3-5)
VS = V // NC    # 1024 vocab shard (step 6)
NCOL = BL * S   # 512 columns of local hT
GHT = 2 * 128 * NCOL          # bf16 elements of hT in gather payload
GLEN = GHT + 128 * BL         # + flattened s

_CACHE: dict = {}


def _build_nc(reps: int = 1):
    import concourse.bacc as bacc
    import concourse.mybir as mybir
    import concourse.tile as tile

    bf = mybir.dt.bfloat16
    f32 = mybir.dt.float32
    AF = mybir.ActivationFunctionType
    ALU = mybir.AluOpType

    nc = bacc.Bacc("TRN2", target_bir_lowering=False, debug=False, num_devices=NC)

    xt = nc.dram_tensor("xt", [V, NCOL], bf, kind="ExternalInput")
    wembT = nc.dram_tensor("wembT", [V, E], bf, kind="ExternalInput")
    bemb = nc.dram_tensor("bemb", [E], f32, kind="ExternalInput")
    wrT = nc.dram_tensor("wrT", [ES, S, E], bf, kind="ExternalInput")
    bredrep = nc.dram_tensor("bredrep", [S, ES], f32, kind="ExternalInput")
    w2p = nc.dram_tensor("w2p", [ES, S, E], bf, kind="ExternalInput")
    bred2 = nc.dram_tensor("bred2", [E], f32, kind="ExternalInput")
    woT = nc.dram_tensor("woT", [E, VS], bf, kind="ExternalInput")
    boutrep = nc.dram_tensor("boutrep", [B, VS], f32, kind="ExternalInput")
    ones = nc.dram_tensor("ones", [S, 1], bf, kind="ExternalInput")
    ident = nc.dram_tensor("ident", [B, B], f32, kind="ExternalInput")
    out_ext = nc.dram_tensor("out", [B, VS], f32, kind="ExternalOutput")

    gin = nc.dram_tensor("gin", [GLEN], bf)
    gout = nc.dram_tensor("gout", [NC, GLEN], bf, addr_space="Shared")
    arin = nc.dram_tensor("arin", [B, E], f32)
    arout = nc.dram_tensor("arout", [B, E], f32, addr_space="Shared")

    groups = [list(range(NC))]

    with tile.TileContext(nc) as tc:
        with (
            tc.tile_pool(name="persist", bufs=1) as pp,
            tc.tile_pool(name="xload", bufs=4) as xpool,
            tc.tile_pool(name="weload", bufs=4) as wepool,
            tc.tile_pool(name="wrload", bufs=4) as wrpool,
            tc.tile_pool(name="w2load", bufs=4) as w2pool,
            tc.tile_pool(name="psum", bufs=1, space="PSUM") as psp,
        ):
            # ---------- persistent SBUF ----------
            hT_all = pp.tile([128, 2 * B * S], bf)       # [j128, (jc, b, s)]
            sT_all = pp.tile([128, B], bf)               # [k, (c,b)]
            weff = pp.tile([128, 2 * ES * B], bf)        # [j128, (jc, e, b)]
            y1 = pp.tile([128, B * ES], bf)              # [k, (b, j)]
            hsb = pp.tile([128, 2 * NCOL], bf)           # local hT [j128,(jc,n)]
            s_bf = pp.tile([1, NCOL], bf)
            bemb_sb = pp.tile([128, 2], f32)
            bredrep_sb = pp.tile([128, ES], f32)
            bred2_sb = pp.tile([128, 2], f32)
            ones_sb = pp.tile([128, 1], bf)
            ident_sb = pp.tile([B, B], f32)
            y2p_sb = pp.tile([B, E], f32)
            y2r_sb = pp.tile([B, E], f32)
            y2T = pp.tile([128, 2 * B], bf)              # [e128, (ec, b)]
            wo_sb = pp.tile([128, 2 * VS], bf)           # [e128, (ec, v)]
            boutrep_sb = pp.tile([B, VS], f32)
            outsb = pp.tile([B, VS], f32)

            nc.sync.dma_start(bemb_sb[:, :], bemb.ap().rearrange("(ec p) -> p ec", p=128))
            nc.sync.dma_start(bredrep_sb[:, :], bredrep[:, :])
            nc.sync.dma_start(bred2_sb[:, :], bred2.ap().rearrange("(ec p) -> p ec", p=128))
            nc.sync.dma_start(ones_sb[:, :], ones[:, :])
            nc.sync.dma_start(ident_sb[:, :], ident[:, :])
            nc.sync.dma_start(boutrep_sb[:, :], boutrep[:, :])
            nc.sync.dma_start(
                wo_sb.rearrange("p (ec v) -> p ec v", ec=2),
                woT.ap().rearrange("(ec p) v -> p ec v", p=128),
            )

            weff_v = weff.rearrange("p (jc e b) -> p jc e b", jc=2, e=ES)
            y1_v = y1.rearrange("p (b j) -> p b j", b=B)

            for _rep in range(reps):
                # ------- phase 1: hT = relu(w_embT.T @ xT + b_emb) -------
                ph0 = psp.tile([128, NCOL], f32, tag="ph0", name="ph0")
                ph1 = psp.tile([128, NCOL], f32, tag="ph1", name="ph1")
                ps = psp.tile([1, NCOL], f32, tag="ps", name="ps")
                phs = [ph0, ph1]
                NV = V // 128
                for vc in range(NV):
                    xt_t = xpool.tile([128, NCOL], bf, tag="xt", name="xt_t")
                    nc.sync.dma_start(xt_t[:, :], xt[vc * 128:(vc + 1) * 128, :])
                    we_t = wepool.tile([128, E], bf, tag="we", name="we_t")
                    nc.sync.dma_start(we_t[:, :], wembT[vc * 128:(vc + 1) * 128, :])
                    for ec in range(2):
                        nc.tensor.matmul(
                            phs[ec][:, :],
                            we_t[:, ec * 128:(ec + 1) * 128],
                            xt_t[:, :],
                            start=(vc == 0),
                            stop=(vc == NV - 1),
                        )
                for ec in range(2):
                    nc.scalar.activation(
                        hsb[:, ec * NCOL:(ec + 1) * NCOL],
                        phs[ec][:, :],
                        AF.Relu,
                        bias=bemb_sb[:, ec:ec + 1],
                    )
                # s = column sums of hT (over both j-chunks)
                for ec in range(2):
                    nc.tensor.matmul(
                        ps[:, :],
                        ones_sb[:, 0:1],
                        hsb[:, ec * NCOL:(ec + 1) * NCOL],
                        start=(ec == 0),
                        stop=(ec == 1),
                    )
                nc.vector.tensor_copy(s_bf[:, :], ps[:, :])

                # ------- gather hT + s across cores -------
                nc.sync.dma_start(
                    gin.ap()[0:GHT].rearrange("(jc p n) -> p jc n", jc=2, p=128),
                    hsb.rearrange("p (jc n) -> p jc n", jc=2),
                )
                nc.sync.dma_start(
                    gin.ap()[GHT:GLEN].rearrange("(one n) -> one n", one=1),
                    s_bf[:, :],
                )
                nc.gpsimd.collective_compute(
                    "AllGather", ALU.bypass, groups,
                    ins=[gin.ap().opt()], outs=[gout.ap().opt()],
                )
                for jc in range(2):
                    nc.sync.dma_start(
                        hT_all[:, jc * B * S:(jc + 1) * B * S].rearrange(
                            "p (c n) -> p c n", c=NC
                        ),
                        gout.ap()[:, jc * 128 * NCOL:(jc + 1) * 128 * NCOL].rearrange(
                            "c (p n) -> p c n", p=128
                        ),
                    )
                for c in range(NC):
                    nc.sync.dma_start(
                        sT_all[:, c * BL:(c + 1) * BL],
                        gout.ap()[c, GHT:GLEN].rearrange("(b k) -> k b", b=BL),
                    )

                # ------- step 3: Weff[b, e, j] = sum_k Wr[e,k,j] s[b,k] -------
                for el in range(ES):
                    wr_t = wrpool.tile([128, E], bf, tag="wr", name="wr_t")
                    nc.sync.dma_start(wr_t[:, :], wrT.ap()[el])
                    for jc in range(2):
                        psw = psp.tile([128, ES], f32, tag="p32", bufs=2, name="psw")
                        nc.tensor.matmul(
                            psw[:, :],
                            wr_t[:, jc * 128:(jc + 1) * 128],
                            sT_all[:, :],
                        )
                        nc.vector.tensor_copy(weff_v[:, jc, el, :], psw[:, :])

                # ------- step 4: y1 = relu(h @ Weff_b^T + b_red) -------
                for b in range(B):
                    psy = psp.tile([128, ES], f32, tag="p32", bufs=2, name="psy")
                    for jc in range(2):
                        nc.tensor.matmul(
                            psy[:, :],
                            hT_all[:, jc * B * S + b * S: jc * B * S + (b + 1) * S],
                            weff_v[:, jc, :, b],
                            start=(jc == 0),
                            stop=(jc == 1),
                        )
                    nc.vector.tensor_tensor(psy[:, :], psy[:, :], bredrep_sb[:, :], ALU.add)
                    nc.scalar.activation(y1_v[:, b, :], psy[:, :], AF.Relu)

                # ------- step 5: y2p[b, eo] = sum_{k, j in shard} -------
                py2 = psp.tile([B, E], f32, tag="py2", name="py2")
                for jl in range(ES):
                    w2_t = w2pool.tile([128, E], bf, tag="w2", name="w2_t")
                    nc.sync.dma_start(w2_t[:, :], w2p.ap()[jl])
                    nc.tensor.matmul(
                        py2[:, :],
                        y1_v[:, :, jl],
                        w2_t[:, :],
                        start=(jl == 0),
                        stop=(jl == ES - 1),
                    )
                nc.vector.tensor_copy(y2p_sb[:, :], py2[:, :])

                # ------- all-reduce partial y2 -------
                nc.sync.dma_start(arin[:, :], y2p_sb[:, :])
                nc.gpsimd.collective_compute(
                    "AllReduce", ALU.add, groups,
                    ins=[arin.ap().opt()], outs=[arout.ap().opt()],
                )
                nc.sync.dma_start(y2r_sb[:, :], arout[:, :])

                # ------- y2T = relu(y2 + b_red2)^T -------
                for ec in range(2):
                    pst = psp.tile([128, B], f32, tag="p32", bufs=2, name="pst")
                    nc.tensor.transpose(
                        pst[:, :], y2r_sb[:, ec * 128:(ec + 1) * 128], ident_sb[:, :]
                    )
                    nc.scalar.activation(
                        y2T[:, ec * B:(ec + 1) * B],
                        pst[:, :],
                        AF.Relu,
                        bias=bred2_sb[:, ec:ec + 1],
                    )

                # ------- step 6: out = y2 @ w_out^T + b_out -------
                for nv in range(2):
                    pso = psp.tile([B, 512], f32, tag="po", bufs=2, name="pso")
                    for ec in range(2):
                        nc.tensor.matmul(
                            pso[:, :],
                            y2T[:, ec * B:(ec + 1) * B],
                            wo_sb[:, ec * VS + nv * 512: ec * VS + (nv + 1) * 512],
                            start=(ec == 0),
                            stop=(ec == 1),
                        )
                    nc.vector.tensor_tensor(
                        outsb[:, nv * 512:(nv + 1) * 512],
                        pso[:, :],
                        boutrep_sb[:, nv * 512:(nv + 1) * 512],
                        ALU.add,
                    )
                nc.sync.dma_start(out_ext[:, :], outsb[:, :])

    nc.compile()
    return nc


def _get_nc():
    if "nc" not in _CACHE:
        _CACHE["nc"] = _build_nc()
    return _CACHE["nc"]


def _pack_inputs(x, w_emb, b_emb, w_red, b_red, w_red2, b_red2, w_out, b_out):
    bf = ml_dtypes.bfloat16
    f32 = np.float32
    wembT = np.ascontiguousarray(w_emb.T).astype(bf)                 # [V, E]
    Wr = np.asarray(w_red).reshape(E, S, E)                          # [e, k, j]
    W2 = np.asarray(w_red2).reshape(E, S, E)                         # [eo, k, j]
    woT = np.ascontiguousarray(np.asarray(w_out).T)                  # [E, V]
    bemb = np.ascontiguousarray(b_emb).astype(f32)
    bred2 = np.ascontiguousarray(b_red2).astype(f32)
    ones = np.ones((S, 1), dtype=bf)
    ident = np.eye(B, dtype=f32)

    in_maps = []
    for c in range(NC):
        xs = np.asarray(x[c * BL:(c + 1) * BL])                      # [4, S, V]
        xt = np.ascontiguousarray(xs.transpose(2, 0, 1).reshape(V, NCOL)).astype(bf)
        wrT_c = np.ascontiguousarray(Wr[c * ES:(c + 1) * ES]).astype(bf)      # [el,k,j]
        w2p_c = np.ascontiguousarray(
            W2[:, :, c * ES:(c + 1) * ES].transpose(2, 1, 0)
        ).astype(bf)                                                  # [jl,k,eo]
        woT_c = np.ascontiguousarray(woT[:, c * VS:(c + 1) * VS]).astype(bf)  # [E,VS]
        bredrep = np.ascontiguousarray(
            np.broadcast_to(b_red[c * ES:(c + 1) * ES], (S, ES))
        ).astype(f32)
        boutrep = np.ascontiguousarray(
            np.broadcast_to(b_out[c * VS:(c + 1) * VS], (B, VS))
        ).astype(f32)
        in_maps.append({
            "xt": xt, "wembT": wembT, "bemb": bemb,
            "wrT": wrT_c, "bredrep": bredrep,
            "w2p": w2p_c, "bred2": bred2,
            "woT": woT_c, "boutrep": boutrep,
            "ones": ones, "ident": ident,
        })
    return in_maps


def kernel(x, w_emb, b_emb, w_red, b_red, w_red2, b_red2, w_out, b_out):
    from concourse.bass_utils import run_bass_kernel_spmd

    nc = _get_nc()
    in_maps = _pack_inputs(x, w_emb, b_emb, w_red, b_red, w_red2, b_red2, w_out, b_out)
    res = run_bass_kernel_spmd(nc, in_maps, core_ids=list(range(NC)))
    out = np.concatenate([res.results[c]["out"] for c in range(NC)], axis=1)
    return np.ascontiguousarray(out, dtype=np.float32)


# revision 18
# speedup vs baseline: 25349.4213x; 2.5971x over previous
"""Trainium2 Bass kernel for nn_CrossBaby_1 (B=32, S=128, V=8192, E=256).

Strategy (8 NeuronCores, single NEFF, collectives):
  - Step 1 (x @ w_emb.T, the 17 GFLOP matmul): data-parallel over batch.
    Each core computes hT for its 4 batches from a host-pretransposed,
    bf16-cast x shard. PSUM-accumulated over 64 K-chunks of V.
  - AllGather of hT (bf16, 256KB/core) + per-batch row sums s.
  - Steps 3-5 (w_red / w_red2, the 67MB of weights): tensor-parallel over
    the e/j feature dim — each core holds 1/8 of w_red and w_red2 and
    processes ALL 32 batches for its feature shard.
  - AllReduce of the partial y2 (32x256 f32).
  - Step 6 (w_out): tensor-parallel over vocab; each core emits
    out[:, c*1024:(c+1)*1024]; host concatenates.
  All matmul operands bf16 (fp32 PSUM accumulation); biases/activations fp32.
"""

import numpy as np
import ml_dtypes

B, S, V, E = 32, 128, 8192, 256
NC = 8
BL = B // NC    # 4 local batches
ES = E // NC    # 32 feature shard (steps 3-5)
VS = V // NC    # 1024 vocab shard (step 6)
NCOL = BL * S   # 512 columns of local hT
GHT = 2 * 128 * NCOL          # bf16 elements of hT in gather payload
GLEN = GHT + 128 * BL         # + flattened s

_CACHE: dict = {}


def _build_nc(reps: int = 1, stop_after: str = "all", skip_cc: bool = False,
              p1_bufs: int = 4, p1_mode: str = "full"):
    import concourse.bacc as bacc
    import concourse.mybir as mybir
    import concourse.tile as tile

    bf = mybir.dt.bfloat16
    f32 = mybir.dt.float32
    AF = mybir.ActivationFunctionType
    ALU = mybir.AluOpType

    nc = bacc.Bacc("TRN2", target_bir_lowering=False, debug=False, num_devices=NC)

    xt = nc.dram_tensor("xt", [V, NCOL], bf, kind="ExternalInput")
    wembT = nc.dram_tensor("wembT", [V, E], bf, kind="ExternalInput")
    bemb = nc.dram_tensor("bemb", [E], f32, kind="ExternalInput")
    wrT = nc.dram_tensor("wrT", [ES, S, E], bf, kind="ExternalInput")
    bredrep = nc.dram_tensor("bredrep", [S, ES], f32, kind="ExternalInput")
    w2p = nc.dram_tensor("w2p", [ES, S, E], bf, kind="ExternalInput")
    bred2 = nc.dram_tensor("bred2", [E], f32, kind="ExternalInput")
    woT = nc.dram_tensor("woT", [E, VS], bf, kind="ExternalInput")
    boutrep = nc.dram_tensor("boutrep", [B, VS], f32, kind="ExternalInput")
    ones = nc.dram_tensor("ones", [S, 1], bf, kind="ExternalInput")
    ident = nc.dram_tensor("ident", [B, B], f32, kind="ExternalInput")
    out_ext = nc.dram_tensor("out", [B, VS], f32, kind="ExternalOutput")

    gin = nc.dram_tensor("gin", [GLEN], bf)
    gout = nc.dram_tensor("gout", [NC, GLEN], bf, addr_space="Shared")
    arin = nc.dram_tensor("arin", [B, E], f32)
    arout = nc.dram_tensor("arout", [B, E], f32, addr_space="Shared")

    groups = [list(range(NC))]

    with tile.TileContext(nc) as tc:
        with (
            tc.tile_pool(name="persist", bufs=1) as pp,
            tc.tile_pool(name="xload", bufs=p1_bufs) as xpool,
            tc.tile_pool(name="weload", bufs=p1_bufs) as wepool,
            tc.tile_pool(name="wrload", bufs=4) as wrpool,
            tc.tile_pool(name="w2load", bufs=4) as w2pool,
            tc.tile_pool(name="psum", bufs=1, space="PSUM") as psp,
        ):
            # ---------- persistent SBUF ----------
            hT_all = pp.tile([128, 2 * B * S], bf)       # [j128, (jc, b, s)]
            sT_all = pp.tile([128, B], bf)               # [k, (c,b)]
            weff = pp.tile([128, 2 * ES * B], bf)        # [j128, (jc, e, b)]
            y1 = pp.tile([128, B * ES], bf)              # [k, (b, j)]
            hsb = pp.tile([128, 2 * NCOL], bf)           # local hT [j128,(jc,n)]
            s_bf = pp.tile([1, NCOL], bf)
            bemb_sb = pp.tile([128, 2], f32)
            bredrep_sb = pp.tile([128, ES], f32)
            bred2_sb = pp.tile([128, 2], f32)
            ones_sb = pp.tile([128, 1], bf)
            ident_sb = pp.tile([B, B], f32)
            y2p_sb = pp.tile([B, E], f32)
            y2r_sb = pp.tile([B, E], f32)
            y2T = pp.tile([128, 2 * B], bf)              # [e128, (ec, b)]
            wo_sb = pp.tile([128, 2 * VS], bf)           # [e128, (ec, v)]
            boutrep_sb = pp.tile([B, VS], f32)
            outsb = pp.tile([B, VS], f32)

            nc.sync.dma_start(bemb_sb[:, :], bemb.ap().rearrange("(ec p) -> p ec", p=128))
            nc.sync.dma_start(bredrep_sb[:, :], bredrep[:, :])
            nc.sync.dma_start(bred2_sb[:, :], bred2.ap().rearrange("(ec p) -> p ec", p=128))
            nc.sync.dma_start(ones_sb[:, :], ones[:, :])
            nc.sync.dma_start(ident_sb[:, :], ident[:, :])
            nc.sync.dma_start(boutrep_sb[:, :], boutrep[:, :])
            nc.sync.dma_start(
                wo_sb.rearrange("p (ec v) -> p ec v", ec=2),
                woT.ap().rearrange("(ec p) v -> p ec v", p=128),
            )

            weff_v = weff.rearrange("p (jc e b) -> p jc e b", jc=2, e=ES)
            y1_v = y1.rearrange("p (b j) -> p b j", b=B)

            _ORD = ["p1", "gather", "p3", "p4", "p5", "ar", "all"]

            def upto(stage):
                return _ORD.index(stage) <= _ORD.index(stop_after)

            for _rep in range(reps):
                # ------- phase 1: hT = relu(w_embT.T @ xT + b_emb) -------
                ph0 = psp.tile([128, NCOL], f32, tag="ph0", name="ph0")
                ph1 = psp.tile([128, NCOL], f32, tag="ph1", name="ph1")
                ps = psp.tile([1, NCOL], f32, tag="ps", name="ps")
                phs = [ph0, ph1]
                NV = V // 128
                first_x = None
                for vc in range(NV):
                    if p1_mode != "mmonly" or first_x is None:
                        xt_t = xpool.tile([128, NCOL], bf, tag="xt", name="xt_t")
                        nc.sync.dma_start(xt_t[:, :], xt[vc * 128:(vc + 1) * 128, :])
                        we_t = wepool.tile([128, E], bf, tag="we", name="we_t")
                        nc.sync.dma_start(we_t[:, :], wembT[vc * 128:(vc + 1) * 128, :])
                        first_x, first_w = xt_t, we_t
                    else:
                        xt_t, we_t = first_x, first_w
                    if p1_mode == "dmaonly":
                        continue
                    for ec in range(2):
                        nc.tensor.matmul(
                            phs[ec][:, :],
                            we_t[:, ec * 128:(ec + 1) * 128],
                            xt_t[:, :],
                            start=(vc == 0),
                            stop=(vc == NV - 1),
                        )
                for ec in range(2):
                    nc.scalar.activation(
                        hsb[:, ec * NCOL:(ec + 1) * NCOL],
                        phs[ec][:, :],
                        AF.Relu,
                        bias=bemb_sb[:, ec:ec + 1],
                    )
                # s = column sums of hT (over both j-chunks)
                for ec in range(2):
                    nc.tensor.matmul(
                        ps[:, :],
                        ones_sb[:, 0:1],
                        hsb[:, ec * NCOL:(ec + 1) * NCOL],
                        start=(ec == 0),
                        stop=(ec == 1),
                    )
                nc.vector.tensor_copy(s_bf[:, :], ps[:, :])

                if not upto("gather"):
                    nc.gpsimd.dma_start(out_ext[:, 0:NCOL], hsb[0:B, 0:NCOL])
                    continue
                # ------- gather hT + s across cores -------
                nc.sync.dma_start(
                    gin.ap()[0:GHT].rearrange("(jc p n) -> p jc n", jc=2, p=128),
                    hsb.rearrange("p (jc n) -> p jc n", jc=2),
                )
                nc.sync.dma_start(
                    gin.ap()[GHT:GLEN].rearrange("(one n) -> one n", one=1),
                    s_bf[:, :],
                )
                if skip_cc:
                    nc.sync.dma_start(gout.ap()[0], gin.ap()[:])
                else:
                    nc.gpsimd.collective_compute(
                        "AllGather", ALU.bypass, groups,
                        ins=[gin.ap().opt()], outs=[gout.ap().opt()],
                    )
                for jc in range(2):
                    nc.sync.dma_start(
                        hT_all[:, jc * B * S:(jc + 1) * B * S].rearrange(
                            "p (c n) -> p c n", c=NC
                        ),
                        gout.ap()[:, jc * 128 * NCOL:(jc + 1) * 128 * NCOL].rearrange(
                            "c (p n) -> p c n", p=128
                        ),
                    )
                for c in range(NC):
                    nc.sync.dma_start(
                        sT_all[:, c * BL:(c + 1) * BL],
                        gout.ap()[c, GHT:GLEN].rearrange("(b k) -> k b", b=BL),
                    )

                if not upto("p3"):
                    nc.gpsimd.dma_start(out_ext[:, 0:B], sT_all[0:B, :])
                    continue
                # ------- step 3: Weff[b, e, j] = sum_k Wr[e,k,j] s[b,k] -------
                for el in range(ES):
                    wr_t = wrpool.tile([128, E], bf, tag="wr", name="wr_t")
                    nc.sync.dma_start(wr_t[:, :], wrT.ap()[el])
                    for jc in range(2):
                        psw = psp.tile([128, ES], f32, tag="p32", bufs=2, name="psw")
                        nc.tensor.matmul(
                            psw[:, :],
                            wr_t[:, jc * 128:(jc + 1) * 128],
                            sT_all[:, :],
                        )
                        nc.vector.tensor_copy(weff_v[:, jc, el, :], psw[:, :])

                if not upto("p4"):
                    nc.gpsimd.dma_start(out_ext[:, 0:64], weff[0:B, 0:64])
                    continue
                # ------- step 4: y1 = relu(h @ Weff_b^T + b_red) -------
                for b in range(B):
                    psy = psp.tile([128, ES], f32, tag="p32", bufs=2, name="psy")
                    for jc in range(2):
                        nc.tensor.matmul(
                            psy[:, :],
                            hT_all[:, jc * B * S + b * S: jc * B * S + (b + 1) * S],
                            weff_v[:, jc, :, b],
                            start=(jc == 0),
                            stop=(jc == 1),
                        )
                    nc.vector.tensor_tensor(psy[:, :], psy[:, :], bredrep_sb[:, :], ALU.add)
                    nc.scalar.activation(y1_v[:, b, :], psy[:, :], AF.Relu)

                if not upto("p5"):
                    nc.gpsimd.dma_start(out_ext[:, 0:64], y1[0:B, 0:64])
                    continue
                # ------- step 5: y2p[b, eo] = sum_{k, j in shard} -------
                py2 = psp.tile([B, E], f32, tag="py2", name="py2")
                for jl in range(ES):
                    w2_t = w2pool.tile([128, E], bf, tag="w2", name="w2_t")
                    nc.sync.dma_start(w2_t[:, :], w2p.ap()[jl])
                    nc.tensor.matmul(
                        py2[:, :],
                        y1_v[:, :, jl],
                        w2_t[:, :],
                        start=(jl == 0),
                        stop=(jl == ES - 1),
                    )
                nc.vector.tensor_copy(y2p_sb[:, :], py2[:, :])

                if not upto("ar"):
                    nc.sync.dma_start(out_ext[:, 0:E], y2p_sb[:, :])
                    continue
                # ------- all-reduce partial y2 -------
                nc.sync.dma_start(arin[:, :], y2p_sb[:, :])
                if skip_cc:
                    nc.sync.dma_start(arout[:, :], arin[:, :])
                else:
                    nc.gpsimd.collective_compute(
                        "AllReduce", ALU.add, groups,
                        ins=[arin.ap().opt()], outs=[arout.ap().opt()],
                    )
                nc.sync.dma_start(y2r_sb[:, :], arout[:, :])

                # ------- y2T = relu(y2 + b_red2)^T -------
                for ec in range(2):
                    pst = psp.tile([128, B], f32, tag="p32", bufs=2, name="pst")
                    nc.tensor.transpose(
                        pst[:, :], y2r_sb[:, ec * 128:(ec + 1) * 128], ident_sb[:, :]
                    )
                    nc.scalar.activation(
                        y2T[:, ec * B:(ec + 1) * B],
                        pst[:, :],
                        AF.Relu,
                        bias=bred2_sb[:, ec:ec + 1],
                    )

                # ------- step 6: out = y2 @ w_out^T + b_out -------
                for nv in range(2):
                    pso = psp.tile([B, 512], f32, tag="po", bufs=2, name="pso")
                    for ec in range(2):
                        nc.tensor.matmul(
                            pso[:, :],
                            y2T[:, ec * B:(ec + 1) * B],
                            wo_sb[:, ec * VS + nv * 512: ec * VS + (nv + 1) * 512],
                            start=(ec == 0),
                            stop=(ec == 1),
                        )
                    nc.vector.tensor_tensor(
                        outsb[:, nv * 512:(nv + 1) * 512],
                        pso[:, :],
                        boutrep_sb[:, nv * 512:(nv + 1) * 512],
                        ALU.add,
                    )
                nc.sync.dma_start(out_ext[:, :], outsb[:, :])

    nc.compile()
    return nc


def _get_nc():
    if "nc" not in _CACHE:
        _CACHE["nc"] = _build_nc()
    return _CACHE["nc"]


def _pack_inputs(x, w_emb, b_emb, w_red, b_red, w_red2, b_red2, w_out, b_out):
    bf = ml_dtypes.bfloat16
    f32 = np.float32
    wembT = np.ascontiguousarray(w_emb.T).astype(bf)                 # [V, E]
    Wr = np.asarray(w_red).reshape(E, S, E)                          # [e, k, j]
    W2 = np.asarray(w_red2).reshape(E, S, E)                         # [eo, k, j]
    woT = np.ascontiguousarray(np.asarray(w_out).T)                  # [E, V]
    bemb = np.ascontiguousarray(b_emb).astype(f32)
    bred2 = np.ascontiguousarray(b_red2).astype(f32)
    ones = np.ones((S, 1), dtype=bf)
    ident = np.eye(B, dtype=f32)

    in_maps = []
    for c in range(NC):
        xs = np.asarray(x[c * BL:(c + 1) * BL])                      # [4, S, V]
        xt = np.ascontiguousarray(xs.transpose(2, 0, 1).reshape(V, NCOL)).astype(bf)
        wrT_c = np.ascontiguousarray(Wr[c * ES:(c + 1) * ES]).astype(bf)      # [el,k,j]
        w2p_c = np.ascontiguousarray(
            W2[:, :, c * ES:(c + 1) * ES].transpose(2, 1, 0)
        ).astype(bf)                                                  # [jl,k,eo]
        woT_c = np.ascontiguousarray(woT[:, c * VS:(c + 1) * VS]).astype(bf)  # [E,VS]
        bredrep = np.ascontiguousarray(
            np.broadcast_to(b_red[c * ES:(c + 1) * ES], (S, ES))
        ).astype(f32)
        boutrep = np.ascontiguousarray(
            np.broadcast_to(b_out[c * VS:(c + 1) * VS], (B, VS))
        ).astype(f32)
        in_maps.append({
            "xt": xt, "wembT": wembT, "bemb": bemb,
            "wrT": wrT_c, "bredrep": bredrep,
            "w2p": w2p_c, "bred2": bred2,
            "woT": woT_c, "boutrep": boutrep,
            "ones": ones, "ident": ident,
        })
    return in_maps


def kernel(x, w_emb, b_emb, w_red, b_red, w_red2, b_red2, w_out, b_out):
    from concourse.bass_utils import run_bass_kernel_spmd

    nc = _get_nc()
    in_maps = _pack_inputs(x, w_emb, b_emb, w_red, b_red, w_red2, b_red2, w_out, b_out)
    res = run_bass_kernel_spmd(nc, in_maps, core_ids=list(range(NC)))
    out = np.concatenate([res.results[c]["out"] for c in range(NC)], axis=1)
    return np.ascontiguousarray(out, dtype=np.float32)
